# revision 12
# baseline (speedup 1.0000x reference)
"""Block-diagonal complex-style locally-connected matmul on 8 NeuronCores.

Math (see reference):
  xp   = x[:, :, perm, :]                  # butterfly permute along N=16384
  xr   = xp[:,0].reshape(B, P, 64)         # P = 4096 blocks, 4*R = 64
  xi   = xp[:,1].reshape(B, P, 64)
  y_re = xr @ W_rr + xi @ W_ri             # per-block [B,64]@[64,64]
  y_im = xr @ W_ir + xi @ W_ii

Device formulation: per block p fold the four 64x64 weights into one
  W_big[p] = [[W_rr, W_ir], [W_ri, W_ii]]  # [128 k, 128 o], k = [xr|xi]
and xcat[b] = [xr|xi]  # [B, 128]; then per block
  y[b, o] = sum_k xcat[b, k] * W_big[k, o]

PE mapping: W_big[p] is the STATIONARY operand ([K=128, M=128] ldweights,
one per block — FWL applies since weights are fp16 and full 128-wide) and
the batch x-slab [K=128, N=8] streams as the moving operand:
  out[o, b] = y[b, o].T  per block, written to ps[:, blk*8 : blk*8+8]
64 blocks fill one PSUM bank [128, 512] DENSELY (no garbage), so the
PSUM->SBUF copy and the out DMA run at full 128-partition width with
contiguous lines, and the out traffic spreads across all 16 SDMA engines.

All HBM streams are fp16 (weights dominate: 16 MB/core), halving traffic
vs f32; accumulation stays fp32 in PSUM so accuracy is ~1e-4.

Sharding: block axis P=4096 split across 8 cores (512 blocks each).
"""

import sys
import types

import numpy as np

import concourse.bass as bass
import concourse.bacc as bacc
import concourse.tile as tile
from concourse import mybir
from concourse.bass_utils import run_bass_kernel_spmd


def _install_ntff_hook_shim():
    """This image's antenv lacks axon_hooks; rebuild it from the boot helper
    so run_bass_kernel_spmd(trace=True) / BASS_TRACE=1 works instead of
    crashing on the missing module."""
    try:
        from antenv.axon_hooks import get_axon_ntff_profile_hook  # noqa: F401

        return
    except ImportError:
        pass
    try:
        from trn_agent_boot.trn_boot import _ntff_profile_via_ctypes

        hook = _ntff_profile_via_ctypes("/opt/axon/libaxon_pjrt.so")
    except Exception:
        hook = None
    mod = types.ModuleType("antenv.axon_hooks")
    mod.get_axon_ntff_profile_hook = lambda: hook
    mod.set_axon_ntff_profile_hook = lambda h: None
    sys.modules["antenv.axon_hooks"] = mod
    try:
        import antenv

        antenv.axon_hooks = mod
    except ImportError:
        pass


_install_ntff_hook_shim()

B = 8
N = 16384
R = 16
P = 4096            # blocks total
NCORES = 8
PC = P // NCORES    # 512 blocks per core
K = 128             # contraction (4*R re + 4*R im)
O = 128             # output features per block (64 re + 64 im)

CHUNK = 32          # blocks per W-chunk DMA (1 MB fp16)
PSB = 32            # blocks per PSUM tile / out-DMA granule

F16 = mybir.dt.float16
F32 = mybir.dt.float32

_NC_CACHE = None


def _build_bass():
    nc = bacc.Bacc(
        "TRN2", target_bir_lowering=False, debug=False, num_devices=NCORES
    )
    w_dram = nc.declare_dram_parameter("wk", [K, PC * O], F16, isOutput=False)
    x_dram = nc.declare_dram_parameter("xk", [K, PC * B], F16, isOutput=False)
    # out[o, p_local*B + b] = y[b, p, o]; host transposes back.
    o_dram = nc.declare_dram_parameter("out", [O, PC * B], F16, isOutput=True)

    n_chunks = PC // CHUNK                     # 16

    with tile.TileContext(nc) as tc:
        with (
            tc.tile_pool(name="wpool", bufs=2) as wpool,
            tc.tile_pool(name="xpool", bufs=1) as xpool,
            tc.tile_pool(name="stg", bufs=3) as stgpool,
            tc.tile_pool(name="ps", bufs=6, space="PSUM") as pspool,
        ):
            # x + out ride the ACT HWDGE ring so their semaphore waits can't
            # head-of-line block W-chunk descriptor generation on the SP ring.
            x_sb = xpool.tile([K, PC * B], F16)
            nc.scalar.dma_start(x_sb[:], x_dram[:])

            for ci in range(n_chunks):
                w_sb = wpool.tile([K, CHUNK * O], F16)
                nc.sync.dma_start(
                    w_sb[:], w_dram[:, ci * CHUNK * O : (ci + 1) * CHUNK * O]
                )
                for half in range(CHUNK // PSB):
                    ps = pspool.tile([K, PSB * B], F32)
                    for j in range(PSB):
                        jj = half * PSB + j
                        p = ci * CHUNK + jj
                        nc.tensor.matmul(
                            ps[:, j * B : (j + 1) * B],
                            w_sb[:, jj * O : (jj + 1) * O],
                            x_sb[:, p * B : (p + 1) * B],
                        )
                    stage = stgpool.tile([K, PSB * B], F16)
                    nc.vector.tensor_copy(stage[:], ps[:])
                    nc.scalar.dma_start(
                        o_dram[
                            :,
                            (ci * CHUNK + half * PSB) * B
                            : (ci * CHUNK + (half + 1) * PSB) * B,
                        ],
                        stage[:],
                    )
    nc.compile()
    return nc


def _get_nc():
    global _NC_CACHE
    if _NC_CACHE is None:
        _NC_CACHE = _build_bass()
    return _NC_CACHE


def _pack_inputs(x, W_rr, W_ri, W_ir, W_ii, perm_idx):
    x = np.asarray(x, dtype=np.float32)
    perm = np.asarray(perm_idx, dtype=np.int64)

    xp = x[:, :, perm, :]                          # [B, 2, N, R]
    xr = xp[:, 0].reshape(B, P, 4 * R)
    xi = xp[:, 1].reshape(B, P, 4 * R)
    xcat = np.concatenate([xr, xi], axis=2)        # [B, P, 128]
    XT = np.ascontiguousarray(
        xcat.transpose(2, 1, 0).astype(np.float16)
    )                                              # [128 k, P, B]

    wtop = np.concatenate([W_rr, W_ir], axis=2)    # [P, 64, 128]
    wbot = np.concatenate([W_ri, W_ii], axis=2)    # [P, 64, 128]
    wbig = np.concatenate([wtop, wbot], axis=1)    # [P, 128 k, 128 o]
    WK = np.ascontiguousarray(
        wbig.transpose(1, 0, 2).astype(np.float16)
    )                                              # [128 k, P, 128 o]

    in_maps = []
    for c in range(NCORES):
        sl = slice(c * PC, (c + 1) * PC)
        in_maps.append(
            {
                "wk": np.ascontiguousarray(WK[:, sl, :]).reshape(K, PC * O),
                "xk": np.ascontiguousarray(XT[:, sl, :]).reshape(K, PC * B),
            }
        )
    return in_maps


def _unpack_outputs(res):
    ycat = np.empty((B, P, O), dtype=np.float32)   # [b, p, o]
    for c in range(NCORES):
        Oc = np.asarray(res.results[c]["out"]).reshape(O, PC, B)
        ycat[:, c * PC : (c + 1) * PC, :] = (
            Oc.transpose(2, 1, 0).astype(np.float32)
        )
    y_re = ycat[:, :, : 4 * R].reshape(B, N, R)
    y_im = ycat[:, :, 4 * R :].reshape(B, N, R)
    y = np.stack([y_re, y_im], axis=1)             # [B, 2, N, R]
    return np.ascontiguousarray(y, dtype=np.float32)


def kernel(x, W_rr, W_ri, W_ir, W_ii, perm_idx):
    in_maps = _pack_inputs(x, W_rr, W_ri, W_ir, W_ii, perm_idx)
    nc = _get_nc()
    res = run_bass_kernel_spmd(nc, in_maps, list(range(NCORES)))
    return _unpack_outputs(res)


# revision 14
# speedup vs baseline: 1.1208x; 1.1208x over previous
"""Block-diagonal complex-style locally-connected matmul on 8 NeuronCores.

Math (see reference):
  xp   = x[:, :, perm, :]                  # butterfly permute along N=16384
  xr   = xp[:,0].reshape(B, P, 64)         # P = 4096 blocks, 4*R = 64
  xi   = xp[:,1].reshape(B, P, 64)
  y_re = xr @ W_rr + xi @ W_ri             # per-block [B,64]@[64,64]
  y_im = xr @ W_ir + xi @ W_ii

Device formulation: per block p fold the four 64x64 weights into one
  W_big[p] = [[W_rr, W_ir], [W_ri, W_ii]]  # [128 k, 128 o], k = [xr|xi]
and xcat[b] = [xr|xi]  # [B, 128]; then per block
  y[b, o] = sum_k xcat[b, k] * W_big[k, o]

PE mapping: W_big[p] is the STATIONARY operand ([K=128, M=128] ldweights,
one per block — FWL applies since weights are fp16 and full 128-wide) and
the batch x-slab [K=128, N=8] streams as the moving operand:
  out[o, b] = y[b, o].T  per block, written to ps[:, blk*8 : blk*8+8]
64 blocks fill one PSUM bank [128, 512] DENSELY (no garbage), so the
PSUM->SBUF copy and the out DMA run at full 128-partition width with
contiguous lines, and the out traffic spreads across all 16 SDMA engines.

All HBM streams are fp16 (weights dominate: 16 MB/core), halving traffic
vs f32; accumulation stays fp32 in PSUM so accuracy is ~1e-4.

Sharding: block axis P=4096 split across 8 cores (512 blocks each).
"""

import sys
import types

import numpy as np

import concourse.bass as bass
import concourse.bacc as bacc
import concourse.tile as tile
from concourse import mybir
from concourse.bass_utils import run_bass_kernel_spmd


def _install_ntff_hook_shim():
    """This image's antenv lacks axon_hooks; rebuild it from the boot helper
    so run_bass_kernel_spmd(trace=True) / BASS_TRACE=1 works instead of
    crashing on the missing module."""
    try:
        from antenv.axon_hooks import get_axon_ntff_profile_hook  # noqa: F401

        return
    except ImportError:
        pass
    try:
        from trn_agent_boot.trn_boot import _ntff_profile_via_ctypes

        hook = _ntff_profile_via_ctypes("/opt/axon/libaxon_pjrt.so")
    except Exception:
        hook = None
    mod = types.ModuleType("antenv.axon_hooks")
    mod.get_axon_ntff_profile_hook = lambda: hook
    mod.set_axon_ntff_profile_hook = lambda h: None
    sys.modules["antenv.axon_hooks"] = mod
    try:
        import antenv

        antenv.axon_hooks = mod
    except ImportError:
        pass


_install_ntff_hook_shim()

B = 8
N = 16384
R = 16
P = 4096            # blocks total
NCORES = 8
PC = P // NCORES    # 512 blocks per core
K = 128             # contraction (4*R re + 4*R im)
O = 128             # output features per block (64 re + 64 im)

CHUNK = 32          # blocks per W-chunk DMA (1 MB fp16)
PSB = 32            # blocks per PSUM tile / out-DMA granule

F16 = mybir.dt.float16
F32 = mybir.dt.float32

_NC_CACHE = None


def _build_bass():
    nc = bacc.Bacc(
        "TRN2", target_bir_lowering=False, debug=False, num_devices=NCORES
    )
    w_dram = nc.declare_dram_parameter("wk", [K, PC * O], F16, isOutput=False)
    x_dram = nc.declare_dram_parameter("xk", [K, PC * B], F16, isOutput=False)
    # out[o, p_local*B + b] = y[b, p, o]; host transposes back.
    o_dram = nc.declare_dram_parameter("out", [O, PC * B], F16, isOutput=True)

    n_chunks = PC // CHUNK                     # 16

    with tile.TileContext(nc) as tc:
        with (
            tc.tile_pool(name="wpool", bufs=10) as wpool,
            tc.tile_pool(name="xpool", bufs=1) as xpool,
            tc.tile_pool(name="stg", bufs=3) as stgpool,
            tc.tile_pool(name="ps", bufs=6, space="PSUM") as pspool,
        ):
            # x + out ride the ACT HWDGE ring so their semaphore waits can't
            # head-of-line block W-chunk descriptor generation on the SP ring.
            x_sb = xpool.tile([K, PC * B], F16)
            nc.scalar.dma_start(x_sb[:], x_dram[:])

            for ci in range(n_chunks):
                w_sb = wpool.tile([K, CHUNK * O], F16)
                nc.sync.dma_start(
                    w_sb[:], w_dram[:, ci * CHUNK * O : (ci + 1) * CHUNK * O]
                )
                # Finer cast/out granules on the final chunk shorten the
                # serial tail after the last W byte lands.
                psb = PSB if ci < n_chunks - 1 else PSB // 2
                for half in range(CHUNK // psb):
                    ps = pspool.tile([K, psb * B], F32)
                    for j in range(psb):
                        jj = half * psb + j
                        p = ci * CHUNK + jj
                        nc.tensor.matmul(
                            ps[:, j * B : (j + 1) * B],
                            w_sb[:, jj * O : (jj + 1) * O],
                            x_sb[:, p * B : (p + 1) * B],
                        )
                    stage = stgpool.tile([K, psb * B], F16)
                    nc.vector.tensor_copy(stage[:], ps[:])
                    nc.scalar.dma_start(
                        o_dram[
                            :,
                            (ci * CHUNK + half * psb) * B
                            : (ci * CHUNK + (half + 1) * psb) * B,
                        ],
                        stage[:],
                    )
    nc.compile()
    return nc


def _get_nc():
    global _NC_CACHE
    if _NC_CACHE is None:
        _NC_CACHE = _build_bass()
    return _NC_CACHE


def _pack_inputs(x, W_rr, W_ri, W_ir, W_ii, perm_idx):
    x = np.asarray(x, dtype=np.float32)
    perm = np.asarray(perm_idx, dtype=np.int64)

    xp = x[:, :, perm, :]                          # [B, 2, N, R]
    xr = xp[:, 0].reshape(B, P, 4 * R)
    xi = xp[:, 1].reshape(B, P, 4 * R)
    xcat = np.concatenate([xr, xi], axis=2)        # [B, P, 128]
    XT = np.ascontiguousarray(
        xcat.transpose(2, 1, 0).astype(np.float16)
    )                                              # [128 k, P, B]

    wtop = np.concatenate([W_rr, W_ir], axis=2)    # [P, 64, 128]
    wbot = np.concatenate([W_ri, W_ii], axis=2)    # [P, 64, 128]
    wbig = np.concatenate([wtop, wbot], axis=1)    # [P, 128 k, 128 o]
    WK = np.ascontiguousarray(
        wbig.transpose(1, 0, 2).astype(np.float16)
    )                                              # [128 k, P, 128 o]

    in_maps = []
    for c in range(NCORES):
        sl = slice(c * PC, (c + 1) * PC)
        in_maps.append(
            {
                "wk": np.ascontiguousarray(WK[:, sl, :]).reshape(K, PC * O),
                "xk": np.ascontiguousarray(XT[:, sl, :]).reshape(K, PC * B),
            }
        )
    return in_maps


def _unpack_outputs(res):
    ycat = np.empty((B, P, O), dtype=np.float32)   # [b, p, o]
    for c in range(NCORES):
        Oc = np.asarray(res.results[c]["out"]).reshape(O, PC, B)
        ycat[:, c * PC : (c + 1) * PC, :] = (
            Oc.transpose(2, 1, 0).astype(np.float32)
        )
    y_re = ycat[:, :, : 4 * R].reshape(B, N, R)
    y_im = ycat[:, :, 4 * R :].reshape(B, N, R)
    y = np.stack([y_re, y_im], axis=1)             # [B, 2, N, R]
    return np.ascontiguousarray(y, dtype=np.float32)


def kernel(x, W_rr, W_ri, W_ir, W_ii, perm_idx):
    in_maps = _pack_inputs(x, W_rr, W_ri, W_ir, W_ii, perm_idx)
    nc = _get_nc()
    res = run_bass_kernel_spmd(nc, in_maps, list(range(NCORES)))
    return _unpack_outputs(res)
